# revision 29
# baseline (speedup 1.0000x reference)
"""GPT2 attention (B=4, S=2048, D=1024, H=16) on 8 trn2 cores.

Sharding: data-parallel over batch (4) x tensor-parallel over heads (2 groups
of 8). Core c handles batch c//2, head group c%2. Each core computes its
partial output projection (row-split c_proj); the host sums the two partials
per batch and adds the (host-folded) biases.

v2 design (vs v1 at ~520us):
  - all matmul inputs bf16 (fp32r streams 2 cyc/row on HW; bf16 1 cyc/row)
  - host supplies x^T so q^T/k^T/v come straight from matmuls (no PE
    transposes, no PSUM transpose copies)
  - QKV for chunk j+1 and projection for chunk j-1 are emitted as PE
    "filler" units interleaved into attention chunk j, so the tensor engine
    never idles while ACT (exp, the 2nd-busiest engine) drains
  - qT kept resident in SBUF (no DRAM spill)
  - softmax denominators via reciprocal_approx_fast (the exact
    nc.vector.reciprocal is ~8 cyc/elem on ONE partition = 4us per call)
  - PSUM budget: scores ring 2x[128,2,512] (4 banks) + attn accum
    1x[65,2,512] (2 banks) + qkv/proj fill pool 2x[128,512] (2 banks)

Per-core kernel phases per sq-chunk j (512 rows):
  qkv(j):  q^T,k^T (feature-major, bias added on PSUM->SBUF cast) into
           resident tiles; v natural with a ones column for softmax sums
  attn(j): scoresT[sk,sq] = kT.T @ qT per head pair (row-packed 64+64),
           pT = exp(scoresT/8) bf16, diagonal masked by upper-tri multiply,
           attnT accumulated over sk blocks via lhsT=[v|1]
  proj(j): out_partial = attnT.T @ c_proj_w (row slice), deferred one chunk
"""
import sys

sys.path.insert(0, "/opt/trn_rl_repo")

import numpy as np
import ml_dtypes
from contextlib import ExitStack

import concourse.bass as bass
import concourse.bacc as bacc
import concourse.mybir as mybir
import concourse.tile as tile
from concourse.masks import make_upper_triangular
from concourse.bass_utils import run_bass_kernel_spmd

F32 = mybir.dt.float32
BF16 = mybir.dt.bfloat16
AF = mybir.ActivationFunctionType
OP = mybir.AluOpType

B, S, D, H = 4, 2048, 1024, 16
DH = 64            # head dim
NCORES = 8
GH = 8             # heads per core
GD = GH * DH       # 512 feature cols per core
ST = S // 128      # 16 s-tiles
KB = D // 128      # 8 contraction blocks
NJ = S // 512      # 4 sq chunks
MT = GD // 128     # 4 m-tiles (= head pairs)
LAG = 4            # PV trails exp by LAG i-blocks


def build_module(repeat=1):
    nc = bacc.Bacc(None, target_bir_lowering=False, debug=False)

    xT = nc.declare_dram_parameter("xT", [D, S], BF16, isOutput=False)
    wq = nc.declare_dram_parameter("wq", [D, GD], BF16, isOutput=False)
    wk = nc.declare_dram_parameter("wk", [D, GD], BF16, isOutput=False)
    wv = nc.declare_dram_parameter("wv", [D, GD], BF16, isOutput=False)
    wp = nc.declare_dram_parameter("wp", [GD, D], BF16, isOutput=False)
    bqk = nc.declare_dram_parameter("bqk", [128, 2 * MT], F32, isOutput=False)
    out = nc.declare_dram_parameter("out", [S, D], F32, isOutput=True)

    with tile.TileContext(nc) as tc, ExitStack() as rctx:
        if repeat > 1:
            rctx.enter_context(tc.For_i(0, repeat, 1))
        _build_body(nc, tc, xT, wq, wk, wv, wp, bqk, out)
    nc.compile()
    return nc


def _build_body(nc, tc, xT, wq, wk, wv, wp, bqk, out):
    with ExitStack() as ctx:
        const = ctx.enter_context(tc.tile_pool(name="const", bufs=1))
        wpool = ctx.enter_context(tc.tile_pool(name="wpool", bufs=1))
        resid = ctx.enter_context(tc.tile_pool(name="resid", bufs=1))
        xch = ctx.enter_context(tc.tile_pool(name="xch", bufs=2))
        pTp = ctx.enter_context(tc.tile_pool(name="pTp", bufs=LAG + 4))
        arw = ctx.enter_context(tc.tile_pool(name="arw", bufs=2))
        rcp = ctx.enter_context(tc.tile_pool(name="rcp", bufs=2))
        rbs = ctx.enter_context(tc.tile_pool(name="rbs", bufs=2))
        ast = ctx.enter_context(tc.tile_pool(name="ast", bufs=16))
        ost = ctx.enter_context(tc.tile_pool(name="ost", bufs=3))
        psc = ctx.enter_context(tc.tile_pool(name="psc", bufs=2, space="PSUM"))
        pat = ctx.enter_context(tc.tile_pool(name="pat", bufs=1, space="PSUM"))
        fill = ctx.enter_context(tc.tile_pool(name="fill", bufs=2, space="PSUM"))

        # ---- constants ----
        tri_f = const.tile([128, 128], F32)  # 1 where col >= row else 0
        make_upper_triangular(nc, tri_f[:], val=1.0, diag=True)
        tri = const.tile([128, 128], BF16)
        nc.vector.tensor_copy(tri[:], tri_f[:])
        ones_v = const.tile([128, ST * GH], BF16)
        nc.gpsimd.memset(ones_v[:], 1.0)
        bqk_sb = const.tile([128, 2 * MT], F32)
        nc.sync.dma_start(bqk_sb[:], bqk.ap())

        # ---- weights (bf16 from host) ----
        wq_sb = [wpool.tile([128, GD], BF16, name=f"wq{k}") for k in range(KB)]
        wk_sb = [wpool.tile([128, GD], BF16, name=f"wk{k}") for k in range(KB)]
        wv_sb = [wpool.tile([128, GD], BF16, name=f"wv{k}") for k in range(KB)]
        wp_sb = [wpool.tile([128, 512], BF16, name=f"wp{i}") for i in range(8)]
        def emit_weight_dmas():
            # queue plan for the startup ramp: sync queue carries xt(0)
            # (emitted first, in make_qkv_units(0)) then wk; gpsimd carries
            # wq then wv. NOTE: scalar-queue DMA triggers execute on the ACT
            # engine and would delay the exp stream -- never use it early.
            for k in range(KB):
                nc.gpsimd.dma_start(
                    wq_sb[k][:], wq.ap()[k * 128:(k + 1) * 128, :])
                nc.sync.dma_start(
                    wk_sb[k][:], wk.ap()[k * 128:(k + 1) * 128, :])
            for k in range(KB):
                nc.gpsimd.dma_start(
                    wv_sb[k][:], wv.ap()[k * 128:(k + 1) * 128, :])
            for k4 in range(4):
                for n in range(2):
                    nc.gpsimd.dma_start(
                        wp_sb[k4 * 2 + n][:],
                        wp.ap()[k4 * 128:(k4 + 1) * 128,
                                n * 512:(n + 1) * 512])

        # ---- residents ----
        kT_sb = [resid.tile([128, S], BF16, name=f"kT{m}") for m in range(MT)]
        qT_sb = [resid.tile([128, S], BF16, name=f"qT{m}") for m in range(MT)]
        # v with ones column at DH, zero-padded to 128 cols per head: a full
        # 128-wide stationary operand enables Fast Weight Load on the PV
        # matmuls (FWL requires NumWeights==128; 66-wide LDW ran ~120ns extra)
        VP = 128
        v_sb = resid.tile([128, ST, GH, VP], BF16)
        nc.vector.tensor_copy(
            v_sb[:, :, :, DH],
            ones_v[:].rearrange("p (a b) -> p a b", a=ST))
        nc.gpsimd.memset(v_sb[:, :, :, DH + 1:VP], 0.0)

        # ---- filler machinery: qkv / proj emitted as PE work units ----
        fillers = []

        def emit_fill(n):
            for _ in range(min(n, len(fillers))):
                fillers.pop(0)()

        def make_qkv_units(j):
            xt = xch.tile([128, KB, 512], BF16, name="xt")
            for k in range(KB):
                nc.sync.dma_start(
                    xt[:, k, :],
                    xT.ap()[k * 128:(k + 1) * 128, j * 512:(j + 1) * 512])
            units = []
            for m in range(MT):
                for wsb, bcol, dst in ((wq_sb, m, qT_sb), (wk_sb, MT + m, kT_sb)):
                    def qk_unit(m=m, wsb=wsb, bcol=bcol, dst=dst):
                        ps = fill.tile([128, 512], F32, name="ps")
                        for k in range(KB):
                            nc.tensor.matmul(
                                ps[:], lhsT=wsb[k][:, m * 128:(m + 1) * 128],
                                rhs=xt[:, k, :],
                                start=(k == 0), stop=(k == KB - 1))
                        nc.vector.tensor_scalar_add(
                            dst[m][:, j * 512:(j + 1) * 512], ps[:],
                            bqk_sb[:, bcol:bcol + 1])
                    units.append(qk_unit)
            for st_i in range(4):
                def v_unit(st_i=st_i):
                    ps = fill.tile([128, 512], F32, name="ps")
                    for k in range(KB):
                        nc.tensor.matmul(
                            ps[:], lhsT=xt[:, k, st_i * 128:(st_i + 1) * 128],
                            rhs=wv_sb[k][:],
                            start=(k == 0), stop=(k == KB - 1))
                    nc.vector.tensor_copy(
                        v_sb[:, 4 * j + st_i, :, 0:DH],
                        ps[:].rearrange("p (h d) -> p h d", h=GH))
                units.append(v_unit)
            return units

        def make_proj_units(j, a_tiles):
            units = []
            for mi4 in range(4):
                for n in range(2):
                    def p_unit(mi4=mi4, n=n):
                        ps = fill.tile([128, 512], F32, name="ps")
                        for k4 in range(4):
                            nc.tensor.matmul(
                                ps[:],
                                lhsT=a_tiles[k4][:, mi4 * 128:(mi4 + 1) * 128],
                                rhs=wp_sb[k4 * 2 + n][:],
                                start=(k4 == 0), stop=(k4 == 3))
                        o_sb = ost.tile([128, 512], F32, name="o_sb")
                        nc.vector.tensor_copy(o_sb[:], ps[:])
                        nc.sync.dma_start(
                            out.ap()[(4 * j + mi4) * 128:(4 * j + mi4 + 1) * 128,
                                     n * 512:(n + 1) * 512], o_sb[:])
                    units.append(p_unit)
            return units

        # prologue: xt(0) DMAs land on the sync queue before the wk DMAs so
        # the first matmuls aren't gated on the weight queue; qkv(0) runs
        # immediately
        qkv0_units = make_qkv_units(0)
        emit_weight_dmas()
        for u in qkv0_units:
            u()

        # ---- main loop: attention(j) with qkv(j+1) as filler; projections
        # of chunks 0..2 are all deferred into attention(3), which otherwise
        # has no qkv filler and runs ACT-bound ----
        late_proj = []
        for j in range(NJ):
            if j + 1 < NJ:
                fillers.extend(make_qkv_units(j + 1))
            else:
                fillers.extend(late_proj)
                late_proj = []
            a_tiles = []
            for p in range(MT):
                if True:  # (high_priority experiment regressed; plain order)
                    at_ps = pat.tile([128, 2, 512], F32, name="at")
                    nlast = 4 * j + 3
                    pv_pend = []   # (i, pT, c0)

                    def emit_pv(i, pT, c0, at_ps=None, nlast=None, p=None):
                        for hh in range(2):
                            nc.tensor.matmul(
                                at_ps[:, hh, c0:],
                                lhsT=v_sb[:, i, 2 * p + hh, :],
                                rhs=pT[:, hh, c0:],
                                start=(i == 0), stop=(i == nlast))

                    for i in range(4 * j + 4):
                        c0 = max(0, i * 128 - j * 512)
                        sc = psc.tile([128, 2, 512], F32, name="sc")
                        for hh in range(2):
                            nc.tensor.matmul(
                                sc[:, hh, c0:],
                                lhsT=kT_sb[p][hh * 64:(hh + 1) * 64,
                                              i * 128:(i + 1) * 128],
                                rhs=qT_sb[p][hh * 64:(hh + 1) * 64,
                                             j * 512 + c0:(j + 1) * 512],
                                start=True, stop=True,
                                tile_position=(hh * 64, 0))
                        pT = pTp.tile([128, 2, 512], BF16, name="pT")
                        nc.scalar.activation(pT[:, :, c0:], sc[:, :, c0:],
                                             AF.Exp, scale=0.125)
                        if i * 128 >= j * 512:  # diagonal: causal mask
                            nc.vector.tensor_tensor(
                                pT[:, :, c0:c0 + 128],
                                pT[:, :, c0:c0 + 128],
                                tri[:, None, :].broadcast_to([128, 2, 128]),
                                op=OP.mult)
                        pv_pend.append((i, pT, c0))
                        if len(pv_pend) > LAG:
                            iq, pq, cq = pv_pend.pop(0)
                            emit_pv(iq, pq, cq, at_ps=at_ps, nlast=nlast, p=p)
                    for iq, pq, cq in pv_pend:
                        emit_pv(iq, pq, cq, at_ps=at_ps, nlast=nlast, p=p)

                    # drain PSUM fast, then normalize by the ones-col row sum
                    a_raw = arw.tile([DH + 1, 2, 512], F32, name="a_raw")
                    nc.vector.tensor_copy(a_raw[:], at_ps[0:DH + 1, :, :])
                    # reciprocal_approx_fast is a custom DVE op that ignores
                    # the AP partition base, so stage the sums row to
                    # partition 0 first (standard ops handle cross-base fine)
                    sums_t = rcp.tile([1, 2, 512], F32, name="sums_t")
                    nc.vector.tensor_copy(sums_t[:], a_raw[DH:DH + 1, :, :])
                    rc = rcp.tile([1, 2, 512], F32, name="rc")
                    nc.vector.reciprocal_approx_fast(rc[:], sums_t[:])
                    a_sb = ast.tile([128, 512], BF16, name="a_sb")
                    for hh in range(2):
                        rb = rbs.tile([64, 512], F32, name="rbsb")
                        nc.gpsimd.partition_broadcast(rb[:], rc[:, hh, :])
                        with nc.allow_low_precision("attn probs; bf16 ok"):
                            nc.vector.tensor_tensor(
                                a_sb[hh * 64:(hh + 1) * 64, :],
                                a_raw[0:DH, hh, :], rb[:], op=OP.mult)
                a_tiles.append(a_sb)
                emit_fill(5)
            if j < NJ - 1:
                late_proj.extend(make_proj_units(j, a_tiles))
            else:
                fillers.extend(make_proj_units(j, a_tiles))
        emit_fill(len(fillers))


_NC = None


def _get_module():
    global _NC
    if _NC is None:
        _NC = build_module()
    return _NC


def make_in_maps(hidden_states, c_attn_w, c_attn_b, c_proj_w):
    bf16 = ml_dtypes.bfloat16
    in_maps = []
    for c in range(NCORES):
        b, g = c // 2, c % 2
        cols = slice(g * GD, (g + 1) * GD)
        bq = np.ascontiguousarray(
            c_attn_b[g * GD:(g + 1) * GD].reshape(MT, 128).T)
        bk = np.ascontiguousarray(
            c_attn_b[D + g * GD:D + (g + 1) * GD].reshape(MT, 128).T)
        in_maps.append({
            "xT": np.ascontiguousarray(hidden_states[b].T).astype(bf16),
            "wq": np.ascontiguousarray(c_attn_w[:, cols]).astype(bf16),
            "wk": np.ascontiguousarray(
                c_attn_w[:, D + g * GD:D + (g + 1) * GD]).astype(bf16),
            "wv": np.ascontiguousarray(
                c_attn_w[:, 2 * D + g * GD:2 * D + (g + 1) * GD]).astype(bf16),
            "wp": np.ascontiguousarray(c_proj_w[g * GD:(g + 1) * GD, :]).astype(bf16),
            "bqk": np.concatenate([bq, bk], axis=1).astype(np.float32),
        })
    return in_maps


def kernel(hidden_states, c_attn_w, c_attn_b, c_proj_w, c_proj_b, _trace=False,
           _tmpdir=None):
    hidden_states = np.asarray(hidden_states, dtype=np.float32)
    c_attn_w = np.asarray(c_attn_w, dtype=np.float32)
    c_attn_b = np.asarray(c_attn_b, dtype=np.float32)
    c_proj_w = np.asarray(c_proj_w, dtype=np.float32)
    c_proj_b = np.asarray(c_proj_b, dtype=np.float32)

    nc = _get_module()
    in_maps = make_in_maps(hidden_states, c_attn_w, c_attn_b, c_proj_w)
    res = run_bass_kernel_spmd(nc, in_maps, list(range(NCORES)), trace=_trace,
                               tmpdir=_tmpdir)

    # v-bias is folded here: attn rows sum to 1, so +b_v passes through the
    # attention average and lands as b_v @ c_proj_w on the output.
    bias_eff = c_proj_b + c_attn_b[2 * D:3 * D] @ c_proj_w
    outp = np.empty((B, S, D), dtype=np.float32)
    for b in range(B):
        outp[b] = (res.results[2 * b]["out"] + res.results[2 * b + 1]["out"]
                   + bias_eff[None, :])
    if _trace:
        return outp, res
    return outp


# revision 30
# speedup vs baseline: 1.0070x; 1.0070x over previous
"""GPT2 attention (B=4, S=2048, D=1024, H=16) on 8 trn2 cores.

Sharding: data-parallel over batch (4) x tensor-parallel over heads (2 groups
of 8). Core c handles batch c//2, head group c%2. Each core computes its
partial output projection (row-split c_proj); the host sums the two partials
per batch and adds the (host-folded) biases.

v2 design (vs v1 at ~520us):
  - all matmul inputs bf16 (fp32r streams 2 cyc/row on HW; bf16 1 cyc/row)
  - host supplies x^T so q^T/k^T/v come straight from matmuls (no PE
    transposes, no PSUM transpose copies)
  - QKV for chunk j+1 and projection for chunk j-1 are emitted as PE
    "filler" units interleaved into attention chunk j, so the tensor engine
    never idles while ACT (exp, the 2nd-busiest engine) drains
  - qT kept resident in SBUF (no DRAM spill)
  - softmax denominators via reciprocal_approx_fast (the exact
    nc.vector.reciprocal is ~8 cyc/elem on ONE partition = 4us per call)
  - PSUM budget: scores ring 2x[128,2,512] (4 banks) + attn accum
    1x[65,2,512] (2 banks) + qkv/proj fill pool 2x[128,512] (2 banks)

Per-core kernel phases per sq-chunk j (512 rows):
  qkv(j):  q^T,k^T (feature-major, bias added on PSUM->SBUF cast) into
           resident tiles; v natural with a ones column for softmax sums
  attn(j): scoresT[sk,sq] = kT.T @ qT per head pair (row-packed 64+64),
           pT = exp(scoresT/8) bf16, diagonal masked by upper-tri multiply,
           attnT accumulated over sk blocks via lhsT=[v|1]
  proj(j): out_partial = attnT.T @ c_proj_w (row slice), deferred one chunk
"""
import sys

sys.path.insert(0, "/opt/trn_rl_repo")

import numpy as np
import ml_dtypes
from contextlib import ExitStack, nullcontext

import concourse.bass as bass
import concourse.bacc as bacc
import concourse.mybir as mybir
import concourse.tile as tile
from concourse.masks import make_upper_triangular
from concourse.bass_utils import run_bass_kernel_spmd

F32 = mybir.dt.float32
BF16 = mybir.dt.bfloat16
AF = mybir.ActivationFunctionType
OP = mybir.AluOpType

B, S, D, H = 4, 2048, 1024, 16
DH = 64            # head dim
NCORES = 8
GH = 8             # heads per core
GD = GH * DH       # 512 feature cols per core
ST = S // 128      # 16 s-tiles
KB = D // 128      # 8 contraction blocks
NJ = S // 512      # 4 sq chunks
MT = GD // 128     # 4 m-tiles (= head pairs)
LAG = 4            # PV trails exp by LAG i-blocks


def build_module(repeat=1):
    nc = bacc.Bacc(None, target_bir_lowering=False, debug=False)

    xT = nc.declare_dram_parameter("xT", [D, S], BF16, isOutput=False)
    wq = nc.declare_dram_parameter("wq", [D, GD], BF16, isOutput=False)
    wk = nc.declare_dram_parameter("wk", [D, GD], BF16, isOutput=False)
    wv = nc.declare_dram_parameter("wv", [D, GD], BF16, isOutput=False)
    wp = nc.declare_dram_parameter("wp", [GD, D], BF16, isOutput=False)
    bqk = nc.declare_dram_parameter("bqk", [128, 2 * MT], F32, isOutput=False)
    out = nc.declare_dram_parameter("out", [S, D], F32, isOutput=True)

    with tile.TileContext(nc) as tc, ExitStack() as rctx:
        if repeat > 1:
            rctx.enter_context(tc.For_i(0, repeat, 1))
        _build_body(nc, tc, xT, wq, wk, wv, wp, bqk, out)
    nc.compile()
    return nc


def _build_body(nc, tc, xT, wq, wk, wv, wp, bqk, out):
    with ExitStack() as ctx:
        const = ctx.enter_context(tc.tile_pool(name="const", bufs=1))
        wpool = ctx.enter_context(tc.tile_pool(name="wpool", bufs=1))
        resid = ctx.enter_context(tc.tile_pool(name="resid", bufs=1))
        xch = ctx.enter_context(tc.tile_pool(name="xch", bufs=2))
        pTp = ctx.enter_context(tc.tile_pool(name="pTp", bufs=LAG + 4))
        arw = ctx.enter_context(tc.tile_pool(name="arw", bufs=2))
        rcp = ctx.enter_context(tc.tile_pool(name="rcp", bufs=2))
        rbs = ctx.enter_context(tc.tile_pool(name="rbs", bufs=2))
        ast = ctx.enter_context(tc.tile_pool(name="ast", bufs=16))
        ost = ctx.enter_context(tc.tile_pool(name="ost", bufs=3))
        psc = ctx.enter_context(tc.tile_pool(name="psc", bufs=2, space="PSUM"))
        pat = ctx.enter_context(tc.tile_pool(name="pat", bufs=1, space="PSUM"))
        fill = ctx.enter_context(tc.tile_pool(name="fill", bufs=2, space="PSUM"))

        # ---- constants ----
        tri_f = const.tile([128, 128], F32)  # 1 where col >= row else 0
        make_upper_triangular(nc, tri_f[:], val=1.0, diag=True)
        tri = const.tile([128, 128], BF16)
        nc.vector.tensor_copy(tri[:], tri_f[:])
        ones_v = const.tile([128, ST * GH], BF16)
        nc.gpsimd.memset(ones_v[:], 1.0)
        bqk_sb = const.tile([128, 2 * MT], F32)
        nc.sync.dma_start(bqk_sb[:], bqk.ap())

        # ---- weights (bf16 from host) ----
        wq_sb = [wpool.tile([128, GD], BF16, name=f"wq{k}") for k in range(KB)]
        wk_sb = [wpool.tile([128, GD], BF16, name=f"wk{k}") for k in range(KB)]
        wv_sb = [wpool.tile([128, GD], BF16, name=f"wv{k}") for k in range(KB)]
        wp_sb = [wpool.tile([128, 512], BF16, name=f"wp{i}") for i in range(8)]
        def emit_weight_dmas():
            # queue plan for the startup ramp: sync queue carries xt(0)
            # (emitted first, in make_qkv_units(0)) then wk; gpsimd carries
            # wq then wv. NOTE: scalar-queue DMA triggers execute on the ACT
            # engine and would delay the exp stream -- never use it early.
            for k in range(KB):
                nc.gpsimd.dma_start(
                    wq_sb[k][:], wq.ap()[k * 128:(k + 1) * 128, :])
                nc.sync.dma_start(
                    wk_sb[k][:], wk.ap()[k * 128:(k + 1) * 128, :])
            for k in range(KB):
                nc.gpsimd.dma_start(
                    wv_sb[k][:], wv.ap()[k * 128:(k + 1) * 128, :])
            for k4 in range(4):
                for n in range(2):
                    nc.gpsimd.dma_start(
                        wp_sb[k4 * 2 + n][:],
                        wp.ap()[k4 * 128:(k4 + 1) * 128,
                                n * 512:(n + 1) * 512])

        # ---- residents ----
        kT_sb = [resid.tile([128, S], BF16, name=f"kT{m}") for m in range(MT)]
        qT_sb = [resid.tile([128, S], BF16, name=f"qT{m}") for m in range(MT)]
        # v with ones column at DH, zero-padded to 128 cols per head: a full
        # 128-wide stationary operand enables Fast Weight Load on the PV
        # matmuls (FWL requires NumWeights==128; 66-wide LDW ran ~120ns extra)
        VP = 128
        v_sb = resid.tile([128, ST, GH, VP], BF16)
        nc.vector.tensor_copy(
            v_sb[:, :, :, DH],
            ones_v[:].rearrange("p (a b) -> p a b", a=ST))
        nc.gpsimd.memset(v_sb[:, :, :, DH + 1:VP], 0.0)

        # ---- filler machinery: qkv / proj emitted as PE work units ----
        fillers = []

        def emit_fill(n):
            for _ in range(min(n, len(fillers))):
                fillers.pop(0)()

        def make_qkv_units(j):
            xt = xch.tile([128, KB, 512], BF16, name="xt")
            for k in range(KB):
                nc.sync.dma_start(
                    xt[:, k, :],
                    xT.ap()[k * 128:(k + 1) * 128, j * 512:(j + 1) * 512])
            units = []
            for m in range(MT):
                for wsb, bcol, dst in ((wq_sb, m, qT_sb), (wk_sb, MT + m, kT_sb)):
                    def qk_unit(m=m, wsb=wsb, bcol=bcol, dst=dst):
                        ps = fill.tile([128, 512], F32, name="ps")
                        for k in range(KB):
                            nc.tensor.matmul(
                                ps[:], lhsT=wsb[k][:, m * 128:(m + 1) * 128],
                                rhs=xt[:, k, :],
                                start=(k == 0), stop=(k == KB - 1))
                        nc.vector.tensor_scalar_add(
                            dst[m][:, j * 512:(j + 1) * 512], ps[:],
                            bqk_sb[:, bcol:bcol + 1])
                    units.append(qk_unit)
            for st_i in range(4):
                def v_unit(st_i=st_i):
                    ps = fill.tile([128, 512], F32, name="ps")
                    for k in range(KB):
                        nc.tensor.matmul(
                            ps[:], lhsT=xt[:, k, st_i * 128:(st_i + 1) * 128],
                            rhs=wv_sb[k][:],
                            start=(k == 0), stop=(k == KB - 1))
                    nc.vector.tensor_copy(
                        v_sb[:, 4 * j + st_i, :, 0:DH],
                        ps[:].rearrange("p (h d) -> p h d", h=GH))
                units.append(v_unit)
            return units

        def make_proj_units(j, a_tiles):
            units = []
            for mi4 in range(4):
                for n in range(2):
                    def p_unit(mi4=mi4, n=n):
                        ps = fill.tile([128, 512], F32, name="ps")
                        for k4 in range(4):
                            nc.tensor.matmul(
                                ps[:],
                                lhsT=a_tiles[k4][:, mi4 * 128:(mi4 + 1) * 128],
                                rhs=wp_sb[k4 * 2 + n][:],
                                start=(k4 == 0), stop=(k4 == 3))
                        o_sb = ost.tile([128, 512], F32, name="o_sb")
                        nc.vector.tensor_copy(o_sb[:], ps[:])
                        nc.sync.dma_start(
                            out.ap()[(4 * j + mi4) * 128:(4 * j + mi4 + 1) * 128,
                                     n * 512:(n + 1) * 512], o_sb[:])
                    units.append(p_unit)
            return units

        # prologue: xt(0) DMAs land on the sync queue before the wk DMAs so
        # the first matmuls aren't gated on the weight queue; qkv(0) runs
        # immediately
        qkv0_units = make_qkv_units(0)
        emit_weight_dmas()
        for u in qkv0_units:
            u()

        # ---- main loop: attention(j) with qkv(j+1) as filler; projections
        # of chunks 0..2 are all deferred into attention(3), which otherwise
        # has no qkv filler and runs ACT-bound ----
        late_proj = []
        for j in range(NJ):
            if j + 1 < NJ:
                fillers.extend(make_qkv_units(j + 1))
            else:
                fillers.extend(late_proj)
                late_proj = []
            a_tiles = []
            for p in range(MT):
                # prime the exp stream: only the first block jumps the
                # filler queue (global high-priority regressed steady-state)
                with (tc.high_priority(offset=1 << 20)
                      if (j == 0 and p == 0) else nullcontext()):
                    at_ps = pat.tile([128, 2, 512], F32, name="at")
                    nlast = 4 * j + 3
                    pv_pend = []   # (i, pT, c0)

                    def emit_pv(i, pT, c0, at_ps=None, nlast=None, p=None):
                        for hh in range(2):
                            nc.tensor.matmul(
                                at_ps[:, hh, c0:],
                                lhsT=v_sb[:, i, 2 * p + hh, :],
                                rhs=pT[:, hh, c0:],
                                start=(i == 0), stop=(i == nlast))

                    for i in range(4 * j + 4):
                        c0 = max(0, i * 128 - j * 512)
                        sc = psc.tile([128, 2, 512], F32, name="sc")
                        for hh in range(2):
                            nc.tensor.matmul(
                                sc[:, hh, c0:],
                                lhsT=kT_sb[p][hh * 64:(hh + 1) * 64,
                                              i * 128:(i + 1) * 128],
                                rhs=qT_sb[p][hh * 64:(hh + 1) * 64,
                                             j * 512 + c0:(j + 1) * 512],
                                start=True, stop=True,
                                tile_position=(hh * 64, 0))
                        pT = pTp.tile([128, 2, 512], BF16, name="pT")
                        nc.scalar.activation(pT[:, :, c0:], sc[:, :, c0:],
                                             AF.Exp, scale=0.125)
                        if i * 128 >= j * 512:  # diagonal: causal mask
                            nc.vector.tensor_tensor(
                                pT[:, :, c0:c0 + 128],
                                pT[:, :, c0:c0 + 128],
                                tri[:, None, :].broadcast_to([128, 2, 128]),
                                op=OP.mult)
                        pv_pend.append((i, pT, c0))
                        if len(pv_pend) > LAG:
                            iq, pq, cq = pv_pend.pop(0)
                            emit_pv(iq, pq, cq, at_ps=at_ps, nlast=nlast, p=p)
                    for iq, pq, cq in pv_pend:
                        emit_pv(iq, pq, cq, at_ps=at_ps, nlast=nlast, p=p)

                    # drain PSUM fast, then normalize by the ones-col row sum
                    a_raw = arw.tile([DH + 1, 2, 512], F32, name="a_raw")
                    nc.vector.tensor_copy(a_raw[:], at_ps[0:DH + 1, :, :])
                    # reciprocal_approx_fast is a custom DVE op that ignores
                    # the AP partition base, so stage the sums row to
                    # partition 0 first (standard ops handle cross-base fine)
                    sums_t = rcp.tile([1, 2, 512], F32, name="sums_t")
                    nc.vector.tensor_copy(sums_t[:], a_raw[DH:DH + 1, :, :])
                    rc = rcp.tile([1, 2, 512], F32, name="rc")
                    nc.vector.reciprocal_approx_fast(rc[:], sums_t[:])
                    a_sb = ast.tile([128, 512], BF16, name="a_sb")
                    for hh in range(2):
                        rb = rbs.tile([64, 512], F32, name="rbsb")
                        nc.gpsimd.partition_broadcast(rb[:], rc[:, hh, :])
                        with nc.allow_low_precision("attn probs; bf16 ok"):
                            nc.vector.tensor_tensor(
                                a_sb[hh * 64:(hh + 1) * 64, :],
                                a_raw[0:DH, hh, :], rb[:], op=OP.mult)
                a_tiles.append(a_sb)
                emit_fill(5)
            if j < NJ - 1:
                late_proj.extend(make_proj_units(j, a_tiles))
            else:
                fillers.extend(make_proj_units(j, a_tiles))
        emit_fill(len(fillers))


_NC = None


def _get_module():
    global _NC
    if _NC is None:
        _NC = build_module()
    return _NC


def make_in_maps(hidden_states, c_attn_w, c_attn_b, c_proj_w):
    bf16 = ml_dtypes.bfloat16
    in_maps = []
    for c in range(NCORES):
        b, g = c // 2, c % 2
        cols = slice(g * GD, (g + 1) * GD)
        bq = np.ascontiguousarray(
            c_attn_b[g * GD:(g + 1) * GD].reshape(MT, 128).T)
        bk = np.ascontiguousarray(
            c_attn_b[D + g * GD:D + (g + 1) * GD].reshape(MT, 128).T)
        in_maps.append({
            "xT": np.ascontiguousarray(hidden_states[b].T).astype(bf16),
            "wq": np.ascontiguousarray(c_attn_w[:, cols]).astype(bf16),
            "wk": np.ascontiguousarray(
                c_attn_w[:, D + g * GD:D + (g + 1) * GD]).astype(bf16),
            "wv": np.ascontiguousarray(
                c_attn_w[:, 2 * D + g * GD:2 * D + (g + 1) * GD]).astype(bf16),
            "wp": np.ascontiguousarray(c_proj_w[g * GD:(g + 1) * GD, :]).astype(bf16),
            "bqk": np.concatenate([bq, bk], axis=1).astype(np.float32),
        })
    return in_maps


def kernel(hidden_states, c_attn_w, c_attn_b, c_proj_w, c_proj_b, _trace=False,
           _tmpdir=None):
    hidden_states = np.asarray(hidden_states, dtype=np.float32)
    c_attn_w = np.asarray(c_attn_w, dtype=np.float32)
    c_attn_b = np.asarray(c_attn_b, dtype=np.float32)
    c_proj_w = np.asarray(c_proj_w, dtype=np.float32)
    c_proj_b = np.asarray(c_proj_b, dtype=np.float32)

    nc = _get_module()
    in_maps = make_in_maps(hidden_states, c_attn_w, c_attn_b, c_proj_w)
    res = run_bass_kernel_spmd(nc, in_maps, list(range(NCORES)), trace=_trace,
                               tmpdir=_tmpdir)

    # v-bias is folded here: attn rows sum to 1, so +b_v passes through the
    # attention average and lands as b_v @ c_proj_w on the output.
    bias_eff = c_proj_b + c_attn_b[2 * D:3 * D] @ c_proj_w
    outp = np.empty((B, S, D), dtype=np.float32)
    for b in range(B):
        outp[b] = (res.results[2 * b]["out"] + res.results[2 * b + 1]["out"]
                   + bias_eff[None, :])
    if _trace:
        return outp, res
    return outp
